# revision 8
# baseline (speedup 1.0000x reference)
"""Trainium2 Bass kernel for a cross-modal transformer block (attention + FFN).

Contract: kernel(**inputs) takes the FULL unsharded inputs (numpy, fp32) and
returns the FULL output [4, 2048, 512] fp32.

Sharding: 8 cores = data-parallel over batch (4) x query-sequence halves (2).
Each core computes K/V projections for its batch's full 2048-token sequence
(cheap duplication) so attention needs no collectives.

Device layout: everything feature-major ([features on partitions, tokens on
free]); the host pre-transposes and pre-casts inputs so the device does zero
transposes.
"""

import functools
import sys

import numpy as np

sys.path.insert(0, "/opt/trn_rl_repo")

import ml_dtypes  # noqa: E402

import concourse.bass as bass  # noqa: E402
import concourse.tile as tile  # noqa: E402
from concourse import bacc, mybir  # noqa: E402
from concourse.bass_utils import run_bass_kernel_spmd  # noqa: E402

BF16 = mybir.dt.bfloat16
F32 = mybir.dt.float32
AF = mybir.ActivationFunctionType
OP = mybir.AluOpType

B, S, D = 4, 2048, 512
H, DH = 8, 64
FF = 2048
P = 128
C = D // P  # 4 feature chunks
CF = FF // P  # 16 ffn chunks
TQ = S // 2  # 1024 query tokens per core
TK = S  # full key sequence per core
KC = TK // P  # 16 key chunks
NT = 512  # token tile (matmul free dim)
NQ = TQ // NT  # 2 query-token tiles
SCALE = 1.0 / np.sqrt(DH)  # 0.125
LN_EPS = 1e-5
NCORES = 8


def _emit(nc, t, es, tc):
    """Emit the per-core program. t: dict name -> DRAM AP."""
    # ---------------- pools ----------------
    wp = es.enter_context(tc.tile_pool(name="w", bufs=1))
    ap_ = es.enter_context(tc.tile_pool(name="acts", bufs=1))
    ptq = es.enter_context(tc.tile_pool(name="ptq", bufs=1))
    psA = es.enter_context(tc.tile_pool(name="psA", bufs=4, space="PSUM"))
    psC = es.enter_context(tc.tile_pool(name="psC", bufs=4, space="PSUM"))
    stream = es.enter_context(tc.tile_pool(name="stream", bufs=6))
    wstream = es.enter_context(tc.tile_pool(name="wstream", bufs=4))
    stage = es.enter_context(tc.tile_pool(name="stage", bufs=1))
    chunk = es.enter_context(tc.tile_pool(name="chunk", bufs=2))
    small = es.enter_context(tc.tile_pool(name="small", bufs=8))
    epool = es.enter_context(tc.tile_pool(name="e", bufs=4))
    hpool = es.enter_context(tc.tile_pool(name="h", bufs=1))

    # ---------------- constants / weights ----------------
    def ld_w(name, kchunks, n):
        w = wp.tile([P, kchunks, n], BF16, name=name + "_sb")
        nc.sync.dma_start(w, t[name].rearrange("(c p) o -> p c o", p=P))
        return w

    wq = ld_w("wq", C, D)
    wk = ld_w("wk", C, D)
    wv = ld_w("wv", C, D)
    wo = ld_w("wo", C, D)
    w1d = t["w1"].rearrange("(c p) o -> p c o", p=P)
    w2d = t["w2"].rearrange("(c p) o -> p c o", p=P)

    def ld_b(name, kchunks):
        b = wp.tile([P, kchunks], F32, name=name + "_sb")
        nc.sync.dma_start(b, t[name].rearrange("(c p) -> p c", p=P))
        return b

    bq = ld_b("bq", C)
    bk = ld_b("bk", C)
    bo = ld_b("bo", C)
    b2 = ld_b("b2", C)
    b1 = ld_b("b1", CF)
    g1 = ld_b("g1", C)
    be1 = ld_b("be1", C)
    g2 = ld_b("g2", C)
    be2 = ld_b("be2", C)

    bvb = wp.tile([P, D], F32)
    nc.gpsimd.dma_start(bvb, t["bv"][None, :].to_broadcast((P, D)))

    ones = wp.tile([P, 1], F32)
    nc.vector.memset(ones, 1.0)
    epst = wp.tile([1, 1], F32)
    nc.vector.memset(epst, LN_EPS)

    # persistent activations (full key sequence)
    kt = ap_.tile([P, C, TK], BF16)  # K.T
    va = ap_.tile([P, KC, H, DH + 1], BF16)  # V token-major, per head + ones col

    nc.vector.memset(va[:, :, :, DH : DH + 1], 1.0)

    xq32d = t["xq32"].rearrange("(c p) q -> p c q", p=P)
    xkb = t["xkb"].rearrange("(c p) q -> p c q", p=P)
    xvb = t["xvb"].rearrange("(c p) q -> p c q", p=P)
    out_d = t["out"].rearrange("(c p) q -> p c q", p=P)

    # ---------------- phase A: K/V projections (full sequence) ----------------
    for tk in range(TK // NT):
        ts_ = slice(tk * NT, (tk + 1) * NT)
        kr = []
        for ki in range(C):
            r = stream.tile([P, NT], BF16, tag="xr", name=f"kr_{tk}_{ki}")
            nc.sync.dma_start(r, xkb[:, ki, ts_])
            kr.append(r)
        for co in range(C):
            ps = psA.tile([P, NT], F32, tag="ps", name=f"kps_{tk}_{co}")
            for ki in range(C):
                nc.tensor.matmul(
                    ps,
                    wk[:, ki, co * P : (co + 1) * P],
                    kr[ki],
                    start=(ki == 0),
                    stop=(ki == C - 1),
                )
            nc.scalar.activation(kt[:, co, ts_], ps, AF.Identity, bias=bk[:, co : co + 1])

    # V projection, token-major out: V = Xv @ Wv  (lhsT = Xv.T chunk)
    for tm in range(KC):
        vl = []
        for ki in range(C):
            r = stream.tile([P, P], BF16, tag="vl", name=f"vl_{tm}_{ki}")
            nc.sync.dma_start(r, xvb[:, ki, tm * P : (tm + 1) * P])
            vl.append(r)
        ps = psA.tile([P, NT], F32, tag="ps", name=f"vps_{tm}")
        for ki in range(C):
            nc.tensor.matmul(ps, vl[ki], wv[:, ki, :], start=(ki == 0), stop=(ki == C - 1))
        for h in range(H):
            nc.vector.tensor_tensor(
                out=va[:, tm, h, 0:DH],
                in0=ps[:, h * DH : (h + 1) * DH],
                in1=bvb[:, h * DH : (h + 1) * DH],
                op=OP.add,
            )

    # ---------------- per query-token-tile: Qproj, attention, tail ----------------
    def layernorm(resid, g, be, out_write):
        """resid: [P, C, NT] f32 tile. out_write(co, t2_f32_tile, be_col)."""
        pm = psA.tile([P, NT], F32, tag="ps", name="ln_pm")
        for co in range(C):
            nc.tensor.matmul(pm[0:1, :], ones, resid[:, co, :], start=(co == 0), stop=(co == C - 1))
        sq = []
        for co in range(C):
            s = chunk.tile([P, NT], F32, tag="sqc", name=f"sq_{co}")
            nc.vector.tensor_mul(s, resid[:, co, :], resid[:, co, :])
            sq.append(s)
        pq = psA.tile([P, NT], F32, tag="ps", name="ln_pq")
        for co in range(C):
            nc.tensor.matmul(pq[0:1, :], ones, sq[co], start=(co == 0), stop=(co == C - 1))
        mean = small.tile([1, NT], F32, tag="sm", name="ln_mean")
        nc.vector.tensor_scalar_mul(mean, pm[0:1, :], 1.0 / D)
        msq = small.tile([1, NT], F32, tag="sm", name="ln_msq")
        nc.vector.tensor_scalar_mul(msq, pq[0:1, :], 1.0 / D)
        m2 = small.tile([1, NT], F32, tag="sm", name="ln_m2")
        nc.vector.tensor_mul(m2, mean, mean)
        var = small.tile([1, NT], F32, tag="sm", name="ln_var")
        nc.vector.tensor_tensor(out=var, in0=msq, in1=m2, op=OP.subtract)
        # rstd = exp(-0.5 * ln(var + eps)) -- stays in the Exp/Ln ACT table set
        lnv = small.tile([1, NT], F32, tag="sm", name="ln_lnv")
        nc.scalar.activation(lnv, var, AF.Ln, bias=epst)
        rstd = small.tile([1, NT], F32, tag="sm", name="ln_rstd")
        nc.scalar.activation(rstd, lnv, AF.Exp, scale=-0.5)
        meanb = chunk.tile([P, NT], F32, tag="bc", name="ln_meanb")
        nc.gpsimd.partition_broadcast(meanb, mean)
        rstdb = chunk.tile([P, NT], F32, tag="bc", name="ln_rstdb")
        nc.gpsimd.partition_broadcast(rstdb, rstd)
        for co in range(C):
            tt = chunk.tile([P, NT], F32, tag="tt", name=f"ln_tt_{co}")
            nc.vector.tensor_tensor(out=tt, in0=resid[:, co, :], in1=meanb, op=OP.subtract)
            t2 = chunk.tile([P, NT], F32, tag="t2", name=f"ln_t2_{co}")
            nc.vector.scalar_tensor_tensor(
                out=t2, in0=tt, scalar=g[:, co : co + 1], in1=rstdb, op0=OP.mult, op1=OP.mult
            )
            out_write(co, t2, be[:, co : co + 1])

    for tq in range(NQ):
        ts_ = slice(tq * NT, (tq + 1) * NT)

        # Q projection for this token tile
        xq32 = stage.tile([P, C, NT], F32, tag="xq32", name=f"xq32_{tq}")
        nc.sync.dma_start(xq32, xq32d[:, :, ts_])
        qt = ptq.tile([P, C, NT], BF16, tag="qt", name=f"qt_{tq}")
        qr = []
        for ki in range(C):
            r = stream.tile([P, NT], BF16, tag="xr", name=f"qr_{tq}_{ki}")
            nc.vector.tensor_copy(out=r, in_=xq32[:, ki, :])
            qr.append(r)
        for co in range(C):
            ps = psA.tile([P, NT], F32, tag="ps", name=f"qps_{tq}_{co}")
            for ki in range(C):
                nc.tensor.matmul(
                    ps,
                    wq[:, ki, co * P : (co + 1) * P],
                    qr[ki],
                    start=(ki == 0),
                    stop=(ki == C - 1),
                )
            nc.scalar.activation(qt[:, co, :], ps, AF.Identity, bias=bq[:, co : co + 1])

        # ---- attention ----
        ctx = ptq.tile([P, C, NT], BF16, tag="ctx", name=f"ctx_{tq}")
        for hp in range(H // 2):  # head pairs sharing a 128-partition chunk
            pc = [
                psC.tile([P, NT], F32, tag="pc", name=f"pc_{tq}_{hp}_{j}")
                for j in range(2)
            ]
            for kc in range(KC):
                ksl = slice(kc * P, (kc + 1) * P)
                for j in range(2):  # head 2*hp + j at partition offset 64*j
                    rows = slice(j * DH, (j + 1) * DH)
                    ps = psA.tile([P, NT], F32, tag="ps", name=f"sps_{tq}_{hp}_{kc}_{j}")
                    # scores.T chunk = K_h @ Q_h.T
                    nc.tensor.matmul(
                        ps, kt[rows, hp, ksl], qt[rows, hp, :], start=True, stop=True
                    )
                    e = epool.tile([P, NT], BF16, tag="e", name=f"e_{tq}_{hp}_{kc}_{j}")
                    nc.scalar.activation(e, ps, AF.Exp, scale=SCALE)
                    # ctx.T (+ sumexp row 64) accumulate:
                    # lhsT = [V_h | 1] token chunk, rhs = E.T chunk
                    nc.tensor.matmul(
                        pc[j][0 : DH + 1, :],
                        va[:, kc, 2 * hp + j, :],
                        e,
                        start=(kc == 0),
                        stop=(kc == KC - 1),
                    )
            for j in range(2):
                rc = small.tile([1, NT], F32, tag="sm", name=f"rc_{tq}_{hp}_{j}")
                nc.vector.reciprocal(rc, pc[j][DH : DH + 1, :])
                db = chunk.tile([DH, NT], F32, tag="db", name=f"db_{tq}_{hp}_{j}")
                nc.gpsimd.partition_broadcast(db, rc)
                nc.vector.tensor_tensor(
                    out=ctx[j * DH : (j + 1) * DH, hp, :],
                    in0=pc[j][0:DH, :],
                    in1=db,
                    op=OP.mult,
                )

        # ---- O projection + residual (query + attn_out) ----
        resid = stage.tile([P, C, NT], F32, tag="resid", name=f"resid_{tq}")
        for co in range(C):
            ps = psA.tile([P, NT], F32, tag="ps", name=f"ops_{tq}_{co}")
            for ki in range(C):
                nc.tensor.matmul(
                    ps,
                    wo[:, ki, co * P : (co + 1) * P],
                    ctx[:, ki, :],
                    start=(ki == 0),
                    stop=(ki == C - 1),
                )
            nc.vector.scalar_tensor_tensor(
                out=resid[:, co, :],
                in0=ps,
                scalar=bo[:, co : co + 1],
                in1=xq32[:, co, :],
                op0=OP.add,
                op1=OP.add,
            )

        # ---- LN1 -> ln1f (f32, kept for resid2) + ln1b (bf16, FFN rhs) ----
        ln1f = stage.tile([P, C, NT], F32, tag="ln1f", name=f"ln1f_{tq}")
        ln1b = ptq.tile([P, C, NT], BF16, tag="ln1b", name=f"ln1b_{tq}")

        def write_ln1(co, t2, bec, ln1f=ln1f, ln1b=ln1b):
            nc.vector.tensor_scalar(
                out=ln1f[:, co, :], in0=t2, scalar1=bec, scalar2=None, op0=OP.add
            )
            nc.vector.tensor_copy(out=ln1b[:, co, :], in_=ln1f[:, co, :])

        layernorm(resid, g1, be1, write_ln1)

        # ---- FFN1 + exact gelu ----
        hb = hpool.tile([P, CF, NT], BF16, tag="h", name=f"h_{tq}")
        for fo in range(CF):
            w1t = wstream.tile([P, C, P], BF16, tag="w1t", name=f"w1t_{tq}_{fo}")
            nc.sync.dma_start(w1t, w1d[:, :, fo * P : (fo + 1) * P])
            ps = psA.tile([P, NT], F32, tag="ps", name=f"fps_{tq}_{fo}")
            for ki in range(C):
                nc.tensor.matmul(
                    ps,
                    w1t[:, ki, :],
                    ln1b[:, ki, :],
                    start=(ki == 0),
                    stop=(ki == C - 1),
                )
            nc.scalar.activation(hb[:, fo, :], ps, AF.Gelu, bias=b1[:, fo : fo + 1])

        # ---- FFN2 + residual2 ----
        resid2 = stage.tile([P, C, NT], F32, tag="resid2", name=f"resid2_{tq}")
        for co in range(C):
            w2t = wstream.tile([P, CF, P], BF16, tag="w2t", name=f"w2t_{tq}_{co}")
            nc.sync.dma_start(w2t, w2d[:, :, co * P : (co + 1) * P])
            ps = psA.tile([P, NT], F32, tag="ps", name=f"gps_{tq}_{co}")
            for ki in range(CF):
                nc.tensor.matmul(
                    ps,
                    w2t[:, ki, :],
                    hb[:, ki, :],
                    start=(ki == 0),
                    stop=(ki == CF - 1),
                )
            nc.vector.scalar_tensor_tensor(
                out=resid2[:, co, :],
                in0=ps,
                scalar=b2[:, co : co + 1],
                in1=ln1f[:, co, :],
                op0=OP.add,
                op1=OP.add,
            )

        # ---- LN2 -> final output chunks -> DRAM ----
        def write_out(co, t2, bec, ts_=ts_):
            oc = chunk.tile([P, NT], F32, tag="oc", name=f"oc_{tq}_{co}")
            nc.vector.tensor_scalar(out=oc, in0=t2, scalar1=bec, scalar2=None, op0=OP.add)
            nc.sync.dma_start(out_d[:, co, ts_], oc)

        layernorm(resid2, g2, be2, write_out)


@functools.lru_cache(maxsize=1)
def build():
    from contextlib import ExitStack

    nc = bacc.Bacc("TRN2", target_bir_lowering=False, debug=False, num_devices=NCORES)
    t = {}

    def din(name, shape, dt):
        t[name] = nc.dram_tensor(name, list(shape), dt, kind="ExternalInput").ap()

    din("xq32", (D, TQ), F32)
    din("xkb", (D, TK), BF16)
    din("xvb", (D, TK), BF16)
    for w in ("wq", "wk", "wv", "wo"):
        din(w, (D, D), BF16)
    din("w1", (D, FF), BF16)
    din("w2", (FF, D), BF16)
    for b in ("bq", "bk", "bv", "bo", "b2", "g1", "be1", "g2", "be2"):
        din(b, (D,), F32)
    din("b1", (FF,), F32)
    t["out"] = nc.dram_tensor("out", [D, TQ], F32, kind="ExternalOutput").ap()

    with tile.TileContext(nc) as tc:
        with ExitStack() as es:
            _emit(nc, t, es, tc)
    nc.compile()
    return nc


def make_in_maps(query, key, value, Wq, bq, Wk, bk, Wv, bv, Wo, bo,
                 g1, be1, g2, be2, W1, b1, W2, b2):
    bf = ml_dtypes.bfloat16
    shared = {
        "wq": np.ascontiguousarray(Wq.astype(bf)),
        "wk": np.ascontiguousarray(Wk.astype(bf)),
        "wv": np.ascontiguousarray(Wv.astype(bf)),
        "wo": np.ascontiguousarray(Wo.astype(bf)),
        "w1": np.ascontiguousarray(W1.astype(bf)),
        "w2": np.ascontiguousarray(W2.astype(bf)),
        "bq": np.asarray(bq, np.float32), "bk": np.asarray(bk, np.float32),
        "bv": np.asarray(bv, np.float32), "bo": np.asarray(bo, np.float32),
        "b1": np.asarray(b1, np.float32), "b2": np.asarray(b2, np.float32),
        "g1": np.asarray(g1, np.float32), "be1": np.asarray(be1, np.float32),
        "g2": np.asarray(g2, np.float32), "be2": np.asarray(be2, np.float32),
    }
    in_maps = []
    for core in range(NCORES):
        b, half = divmod(core, 2)
        qsl = slice(half * TQ, (half + 1) * TQ)
        xq_t = np.ascontiguousarray(np.asarray(query[b, qsl], np.float32).T)
        xk_t = np.ascontiguousarray(np.asarray(key[b], np.float32).T.astype(bf))
        xv_t = np.ascontiguousarray(np.asarray(value[b], np.float32).T.astype(bf))
        in_maps.append({"xq32": xq_t, "xkb": xk_t, "xvb": xv_t, **shared})
    return in_maps


def kernel(**inputs):
    nc = build()
    in_maps = make_in_maps(**inputs)
    res = run_bass_kernel_spmd(nc, in_maps, list(range(NCORES)))
    out = np.empty((B, S, D), np.float32)
    for core in range(NCORES):
        b, half = divmod(core, 2)
        out[b, half * TQ : (half + 1) * TQ] = res.results[core]["out"].T
    return out


if __name__ == "__main__":
    import reference

    inputs = {k: np.asarray(v) for k, v in reference.setup_inputs().items()}
    got = kernel(**inputs)
    exp = np.asarray(reference.reference(**inputs))
    err = np.abs(got - exp).max() / np.abs(exp).max()
    print("rel err:", err)


# revision 14
# speedup vs baseline: 1.0596x; 1.0596x over previous
"""Trainium2 Bass kernel for a cross-modal transformer block (attention + FFN).

Contract: kernel(**inputs) takes the FULL unsharded inputs (numpy, fp32) and
returns the FULL output [4, 2048, 512] fp32.

Sharding: 8 cores = data-parallel over batch (4) x query-sequence halves (2).
Each core computes K/V projections for its batch's full 2048-token sequence
(cheap duplication) so attention needs no collectives.

Device layout: everything feature-major ([features on partitions, tokens on
free]); the host pre-transposes and pre-casts inputs so the device does zero
transposes.
"""

import functools
import sys

import numpy as np

sys.path.insert(0, "/opt/trn_rl_repo")

import ml_dtypes  # noqa: E402

import concourse.bass as bass  # noqa: E402
import concourse.tile as tile  # noqa: E402
from concourse import bacc, mybir  # noqa: E402
from concourse.bass_utils import run_bass_kernel_spmd  # noqa: E402

BF16 = mybir.dt.bfloat16
F32 = mybir.dt.float32
AF = mybir.ActivationFunctionType
OP = mybir.AluOpType

B, S, D = 4, 2048, 512
H, DH = 8, 64
FF = 2048
P = 128
C = D // P  # 4 feature chunks
CF = FF // P  # 16 ffn chunks
TQ = S // 2  # 1024 query tokens per core
TK = S  # full key sequence per core
KC = TK // P  # 16 key chunks
NT = 512  # token tile (matmul free dim)
NQ = TQ // NT  # 2 query-token tiles
SCALE = 1.0 / np.sqrt(DH)  # 0.125
LN_EPS = 1e-5
NCORES = 8


def _emit(nc, t, es, tc):
    """Emit the per-core program. t: dict name -> DRAM AP."""
    # ---------------- pools ----------------
    wp = es.enter_context(tc.tile_pool(name="w", bufs=1))
    ap_ = es.enter_context(tc.tile_pool(name="acts", bufs=1))
    ptq = es.enter_context(tc.tile_pool(name="ptq", bufs=1))
    psA = es.enter_context(tc.tile_pool(name="psA", bufs=2, space="PSUM"))
    psS = es.enter_context(tc.tile_pool(name="psS", bufs=2, space="PSUM"))
    psC = es.enter_context(tc.tile_pool(name="psC", bufs=2, space="PSUM"))
    stream = es.enter_context(tc.tile_pool(name="stream", bufs=6))
    wstream = es.enter_context(tc.tile_pool(name="wstream", bufs=4))
    stage = es.enter_context(tc.tile_pool(name="stage", bufs=1))
    chunk = es.enter_context(tc.tile_pool(name="chunk", bufs=2))
    small = es.enter_context(tc.tile_pool(name="small", bufs=8))
    epool = es.enter_context(tc.tile_pool(name="e", bufs=4))
    hpool = es.enter_context(tc.tile_pool(name="h", bufs=1))

    # ---------------- constants / weights ----------------
    def ld_w(name, kchunks, n):
        w = wp.tile([P, kchunks, n], BF16, name=name + "_sb")
        nc.sync.dma_start(w, t[name].rearrange("(c p) o -> p c o", p=P))
        return w

    def ld_b(name, kchunks):
        b = wp.tile([P, kchunks], F32, name=name + "_sb")
        nc.sync.dma_start(b, t[name].rearrange("(c p) -> p c", p=P))
        return b

    # load order: K-proj operands first so PE work starts ASAP
    wk = ld_w("wk", C, D)
    bk = ld_b("bk", C)
    wv = ld_w("wv", C, D)
    wq = ld_w("wq", C, D)
    wo = ld_w("wo", C, D)
    w1d = t["w1"].rearrange("(c p) o -> p c o", p=P)
    w2d = t["w2"].rearrange("(c p) o -> p c o", p=P)

    bq = ld_b("bq", C)
    bo = ld_b("bo", C)
    b2 = ld_b("b2", C)
    b1 = ld_b("b1", CF)
    g1 = ld_b("g1", C)
    be1 = ld_b("be1", C)
    g2 = ld_b("g2", C)
    be2 = ld_b("be2", C)

    bvb = wp.tile([P, D], F32)
    nc.gpsimd.dma_start(bvb, t["bv"][None, :].to_broadcast((P, D)))

    ones = wp.tile([P, 1], F32)
    nc.vector.memset(ones, 1.0)
    epst = wp.tile([1, 1], F32)
    nc.vector.memset(epst, LN_EPS)

    # persistent activations (full key sequence)
    kt = ap_.tile([P, C, TK], BF16)  # K.T
    va = ap_.tile([P, KC, H, DH + 1], BF16)  # V token-major, per head + ones col

    nc.vector.memset(va[:, :, :, DH : DH + 1], 1.0)

    xq32d = t["xq32"].rearrange("(c p) q -> p c q", p=P)
    xkb = t["xkb"].rearrange("(c p) q -> p c q", p=P)
    xvb = t["xvb"].rearrange("(c p) q -> p c q", p=P)
    out_d = t["out"].rearrange("(c p) q -> p c q", p=P)

    # ---------------- phase A: K/V projections (full sequence) ----------------
    for tk in range(TK // NT):
        ts_ = slice(tk * NT, (tk + 1) * NT)
        kr = []
        for ki in range(C):
            r = stream.tile([P, NT], BF16, tag="xr", name=f"kr_{tk}_{ki}")
            nc.sync.dma_start(r, xkb[:, ki, ts_])
            kr.append(r)
        for co in range(C):
            ps = psA.tile([P, NT], F32, tag="ps", name=f"kps_{tk}_{co}")
            for ki in range(C):
                nc.tensor.matmul(
                    ps,
                    wk[:, ki, co * P : (co + 1) * P],
                    kr[ki],
                    start=(ki == 0),
                    stop=(ki == C - 1),
                )
            nc.vector.tensor_scalar(
                out=kt[:, co, ts_], in0=ps, scalar1=bk[:, co : co + 1],
                scalar2=None, op0=OP.add,
            )

    # V projection, token-major out: V = Xv @ Wv  (lhsT = Xv.T chunk)
    for tm in range(KC):
        vl = []
        for ki in range(C):
            r = stream.tile([P, P], BF16, tag="vl", name=f"vl_{tm}_{ki}")
            nc.sync.dma_start(r, xvb[:, ki, tm * P : (tm + 1) * P])
            vl.append(r)
        ps = psA.tile([P, NT], F32, tag="ps", name=f"vps_{tm}")
        for ki in range(C):
            nc.tensor.matmul(ps, vl[ki], wv[:, ki, :], start=(ki == 0), stop=(ki == C - 1))
        for h in range(H):
            nc.vector.tensor_tensor(
                out=va[:, tm, h, 0:DH],
                in0=ps[:, h * DH : (h + 1) * DH],
                in1=bvb[:, h * DH : (h + 1) * DH],
                op=OP.add,
            )

    # ---------------- per query-token-tile: Qproj, attention, tail ----------------
    def layernorm(resid, g, be, out_write):
        """resid: [P, C, NT] f32 tile. out_write(co, t2_f32_tile, be_col)."""
        pm = psA.tile([P, NT], F32, tag="ps", name="ln_pm")
        for co in range(C):
            nc.tensor.matmul(pm[0:1, :], ones, resid[:, co, :], start=(co == 0), stop=(co == C - 1))
        sq = []
        for co in range(C):
            s = chunk.tile([P, NT], F32, tag="sqc", name=f"sq_{co}")
            nc.vector.tensor_mul(s, resid[:, co, :], resid[:, co, :])
            sq.append(s)
        pq = psA.tile([P, NT], F32, tag="ps", name="ln_pq")
        for co in range(C):
            nc.tensor.matmul(pq[0:1, :], ones, sq[co], start=(co == 0), stop=(co == C - 1))
        mean = small.tile([1, NT], F32, tag="sm", name="ln_mean")
        nc.vector.tensor_scalar_mul(mean, pm[0:1, :], 1.0 / D)
        msq = small.tile([1, NT], F32, tag="sm", name="ln_msq")
        nc.vector.tensor_scalar_mul(msq, pq[0:1, :], 1.0 / D)
        m2 = small.tile([1, NT], F32, tag="sm", name="ln_m2")
        nc.vector.tensor_mul(m2, mean, mean)
        var = small.tile([1, NT], F32, tag="sm", name="ln_var")
        nc.vector.tensor_tensor(out=var, in0=msq, in1=m2, op=OP.subtract)
        # rstd = exp(-0.5 * ln(var + eps)) -- stays in the Exp/Ln ACT table set
        lnv = small.tile([1, NT], F32, tag="sm", name="ln_lnv")
        nc.scalar.activation(lnv, var, AF.Ln, bias=epst)
        rstd = small.tile([1, NT], F32, tag="sm", name="ln_rstd")
        nc.scalar.activation(rstd, lnv, AF.Exp, scale=-0.5)
        meanb = chunk.tile([P, NT], F32, tag="bc", name="ln_meanb")
        nc.gpsimd.partition_broadcast(meanb, mean)
        rstdb = chunk.tile([P, NT], F32, tag="bc", name="ln_rstdb")
        nc.gpsimd.partition_broadcast(rstdb, rstd)
        for co in range(C):
            tt = chunk.tile([P, NT], F32, tag="tt", name=f"ln_tt_{co}")
            nc.vector.tensor_tensor(out=tt, in0=resid[:, co, :], in1=meanb, op=OP.subtract)
            t2 = chunk.tile([P, NT], F32, tag="t2", name=f"ln_t2_{co}")
            nc.vector.scalar_tensor_tensor(
                out=t2, in0=tt, scalar=g[:, co : co + 1], in1=rstdb, op0=OP.mult, op1=OP.mult
            )
            out_write(co, t2, be[:, co : co + 1])

    for tq in range(NQ):
        ts_ = slice(tq * NT, (tq + 1) * NT)

        # Q projection for this token tile
        xq32 = stage.tile([P, C, NT], F32, tag="xq32", name=f"xq32_{tq}")
        nc.sync.dma_start(xq32, xq32d[:, :, ts_])
        qt = ptq.tile([P, C, NT], BF16, tag="qt", name=f"qt_{tq}")
        qr = []
        for ki in range(C):
            r = stream.tile([P, NT], BF16, tag="xr", name=f"qr_{tq}_{ki}")
            nc.vector.tensor_copy(out=r, in_=xq32[:, ki, :])
            qr.append(r)
        for co in range(C):
            ps = psA.tile([P, NT], F32, tag="ps", name=f"qps_{tq}_{co}")
            for ki in range(C):
                nc.tensor.matmul(
                    ps,
                    wq[:, ki, co * P : (co + 1) * P],
                    qr[ki],
                    start=(ki == 0),
                    stop=(ki == C - 1),
                )
            nc.vector.tensor_scalar(
                out=qt[:, co, :], in0=ps, scalar1=bq[:, co : co + 1],
                scalar2=None, op0=OP.add,
            )

        # ---- attention ----
        ctx = ptq.tile([P, C, NT], BF16, tag="ctx", name=f"ctx_{tq}")
        for hp in range(H // 2):  # head pairs sharing a 128-partition chunk
            pc = [
                psC.tile([P, NT], F32, tag="pc", name=f"pc_{tq}_{hp}_{j}")
                for j in range(2)
            ]
            for kc in range(KC):
                ksl = slice(kc * P, (kc + 1) * P)
                # both heads' scores into one 2-bank PSUM tile -> one big exp
                ps2 = psS.tile([P, 2, NT], F32, tag="ps2", name=f"sps_{tq}_{hp}_{kc}")
                e2 = epool.tile([P, 2, NT], BF16, tag="e", name=f"e_{tq}_{hp}_{kc}")
                for j in range(2):  # head 2*hp + j at partition offset 64*j
                    rows = slice(j * DH, (j + 1) * DH)
                    # scores.T chunk = K_h @ Q_h.T
                    nc.tensor.matmul(
                        ps2[:, j, :], kt[rows, hp, ksl], qt[rows, hp, :],
                        start=True, stop=True,
                    )
                nc.scalar.activation(e2, ps2, AF.Exp, scale=SCALE)
                for j in range(2):
                    # ctx.T (+ sumexp row 64) accumulate:
                    # lhsT = [V_h | 1] token chunk, rhs = E.T chunk
                    nc.tensor.matmul(
                        pc[j][0 : DH + 1, :],
                        va[:, kc, 2 * hp + j, :],
                        e2[:, j, :],
                        start=(kc == 0),
                        stop=(kc == KC - 1),
                    )
            for j in range(2):
                # copy out of PSUM promptly so the accumulator bank frees for
                # the next head pair; normalize from SBUF off the critical path
                rc = small.tile([1, NT], F32, tag="sm", name=f"rc_{tq}_{hp}_{j}")
                nc.vector.reciprocal(rc, pc[j][DH : DH + 1, :])
                cf = chunk.tile([DH, NT], F32, tag="cf", name=f"cf_{tq}_{hp}_{j}")
                nc.vector.tensor_copy(out=cf, in_=pc[j][0:DH, :])
                db = chunk.tile([DH, NT], F32, tag="db", name=f"db_{tq}_{hp}_{j}")
                nc.gpsimd.partition_broadcast(db, rc)
                nc.vector.tensor_tensor(
                    out=ctx[j * DH : (j + 1) * DH, hp, :],
                    in0=cf,
                    in1=db,
                    op=OP.mult,
                )

        # ---- O projection + residual (query + attn_out) ----
        resid = stage.tile([P, C, NT], F32, tag="resid", name=f"resid_{tq}")
        for co in range(C):
            ps = psA.tile([P, NT], F32, tag="ps", name=f"ops_{tq}_{co}")
            for ki in range(C):
                nc.tensor.matmul(
                    ps,
                    wo[:, ki, co * P : (co + 1) * P],
                    ctx[:, ki, :],
                    start=(ki == 0),
                    stop=(ki == C - 1),
                )
            nc.vector.scalar_tensor_tensor(
                out=resid[:, co, :],
                in0=ps,
                scalar=bo[:, co : co + 1],
                in1=xq32[:, co, :],
                op0=OP.add,
                op1=OP.add,
            )

        # ---- LN1 -> ln1f (f32, kept for resid2) + ln1b (bf16, FFN rhs) ----
        ln1f = stage.tile([P, C, NT], F32, tag="ln1f", name=f"ln1f_{tq}")
        ln1b = ptq.tile([P, C, NT], BF16, tag="ln1b", name=f"ln1b_{tq}")

        def write_ln1(co, t2, bec, ln1f=ln1f, ln1b=ln1b):
            nc.vector.tensor_scalar(
                out=ln1f[:, co, :], in0=t2, scalar1=bec, scalar2=None, op0=OP.add
            )
            nc.vector.tensor_copy(out=ln1b[:, co, :], in_=ln1f[:, co, :])

        layernorm(resid, g1, be1, write_ln1)

        # ---- FFN1 + exact gelu ----
        hb = hpool.tile([P, CF, NT], BF16, tag="h", name=f"h_{tq}")
        for fo in range(CF):
            w1t = wstream.tile([P, C, P], BF16, tag="w1t", name=f"w1t_{tq}_{fo}")
            nc.sync.dma_start(w1t, w1d[:, :, fo * P : (fo + 1) * P])
            ps = psA.tile([P, NT], F32, tag="ps", name=f"fps_{tq}_{fo}")
            for ki in range(C):
                nc.tensor.matmul(
                    ps,
                    w1t[:, ki, :],
                    ln1b[:, ki, :],
                    start=(ki == 0),
                    stop=(ki == C - 1),
                )
            nc.scalar.activation(hb[:, fo, :], ps, AF.Gelu, bias=b1[:, fo : fo + 1])

        # ---- FFN2 + residual2 ----
        resid2 = stage.tile([P, C, NT], F32, tag="resid2", name=f"resid2_{tq}")
        for co in range(C):
            w2t = wstream.tile([P, CF, P], BF16, tag="w2t", name=f"w2t_{tq}_{co}")
            nc.sync.dma_start(w2t, w2d[:, :, co * P : (co + 1) * P])
            ps = psA.tile([P, NT], F32, tag="ps", name=f"gps_{tq}_{co}")
            for ki in range(CF):
                nc.tensor.matmul(
                    ps,
                    w2t[:, ki, :],
                    hb[:, ki, :],
                    start=(ki == 0),
                    stop=(ki == CF - 1),
                )
            nc.vector.scalar_tensor_tensor(
                out=resid2[:, co, :],
                in0=ps,
                scalar=b2[:, co : co + 1],
                in1=ln1f[:, co, :],
                op0=OP.add,
                op1=OP.add,
            )

        # ---- LN2 -> final output chunks -> DRAM ----
        def write_out(co, t2, bec, ts_=ts_):
            oc = chunk.tile([P, NT], F32, tag="oc", name=f"oc_{tq}_{co}")
            nc.vector.tensor_scalar(out=oc, in0=t2, scalar1=bec, scalar2=None, op0=OP.add)
            nc.sync.dma_start(out_d[:, co, ts_], oc)

        layernorm(resid2, g2, be2, write_out)


@functools.lru_cache(maxsize=1)
def build():
    from contextlib import ExitStack

    nc = bacc.Bacc("TRN2", target_bir_lowering=False, debug=False, num_devices=NCORES)
    t = {}

    def din(name, shape, dt):
        t[name] = nc.dram_tensor(name, list(shape), dt, kind="ExternalInput").ap()

    din("xq32", (D, TQ), F32)
    din("xkb", (D, TK), BF16)
    din("xvb", (D, TK), BF16)
    for w in ("wq", "wk", "wv", "wo"):
        din(w, (D, D), BF16)
    din("w1", (D, FF), BF16)
    din("w2", (FF, D), BF16)
    for b in ("bq", "bk", "bv", "bo", "b2", "g1", "be1", "g2", "be2"):
        din(b, (D,), F32)
    din("b1", (FF,), F32)
    t["out"] = nc.dram_tensor("out", [D, TQ], F32, kind="ExternalOutput").ap()

    with tile.TileContext(nc) as tc:
        with ExitStack() as es:
            _emit(nc, t, es, tc)
    nc.compile()
    return nc


def make_in_maps(query, key, value, Wq, bq, Wk, bk, Wv, bv, Wo, bo,
                 g1, be1, g2, be2, W1, b1, W2, b2):
    bf = ml_dtypes.bfloat16
    shared = {
        "wq": np.ascontiguousarray(Wq.astype(bf)),
        "wk": np.ascontiguousarray(Wk.astype(bf)),
        "wv": np.ascontiguousarray(Wv.astype(bf)),
        "wo": np.ascontiguousarray(Wo.astype(bf)),
        "w1": np.ascontiguousarray(W1.astype(bf)),
        "w2": np.ascontiguousarray(W2.astype(bf)),
        "bq": np.asarray(bq, np.float32), "bk": np.asarray(bk, np.float32),
        "bv": np.asarray(bv, np.float32), "bo": np.asarray(bo, np.float32),
        "b1": np.asarray(b1, np.float32), "b2": np.asarray(b2, np.float32),
        "g1": np.asarray(g1, np.float32), "be1": np.asarray(be1, np.float32),
        "g2": np.asarray(g2, np.float32), "be2": np.asarray(be2, np.float32),
    }
    in_maps = []
    for core in range(NCORES):
        b, half = divmod(core, 2)
        qsl = slice(half * TQ, (half + 1) * TQ)
        xq_t = np.ascontiguousarray(np.asarray(query[b, qsl], np.float32).T)
        xk_t = np.ascontiguousarray(np.asarray(key[b], np.float32).T.astype(bf))
        xv_t = np.ascontiguousarray(np.asarray(value[b], np.float32).T.astype(bf))
        in_maps.append({"xq32": xq_t, "xkb": xk_t, "xvb": xv_t, **shared})
    return in_maps


def kernel(**inputs):
    nc = build()
    in_maps = make_in_maps(**inputs)
    res = run_bass_kernel_spmd(nc, in_maps, list(range(NCORES)))
    out = np.empty((B, S, D), np.float32)
    for core in range(NCORES):
        b, half = divmod(core, 2)
        out[b, half * TQ : (half + 1) * TQ] = res.results[core]["out"].T
    return out


if __name__ == "__main__":
    import reference

    inputs = {k: np.asarray(v) for k, v in reference.setup_inputs().items()}
    got = kernel(**inputs)
    exp = np.asarray(reference.reference(**inputs))
    err = np.abs(got - exp).max() / np.abs(exp).max()
    print("rel err:", err)
